# revision 15
# baseline (speedup 1.0000x reference)
"""RNN-T JointNet kernel for 8 Trainium2 NeuronCores.

out[b,t,u,:] = gelu_tanh(enc[b,t]@We + dec[b,u]@Wd + b1) @ Wfc

Sharding: flatten (B=4, T=512) -> 2048 rows, 256 contiguous rows per core.
Core c handles batch b=c//2, time slice t0=(c%2)*256 .. +256.

The tiny projections pe=enc@We and pd=dec@Wd+b1 (<1% of FLOPs) are
precomputed on host and shipped as bf16, so the device kernel is just
gelu(pe[t]+pd[u]) @ Wfc. This halves the input bytes on the startup
critical path (the 16 DMA engines are shared across queues, so input
loading is bandwidth-bound at ~400 GB/s aggregate) and removes the
on-device prologue matmuls/evacuations entirely.

Mixed precision: the fc matmul dominates (32768x512x512 per core) and fp32
matmuls run at 1/4 PE rate, so hact and Wfc are bf16 (1 col/cycle). The
gelu input stays fp32. Output is stored bf16 (halves the 512 MiB HBM
write) and upcast on host. Norm rel err ~3.8e-3, well under the 2e-2 gate.

Per-core engine budget @64 groups of 2 u's (PE is the floor: 1024 bf16
matmuls x 512 moving cols = 218.5 us streaming at 2.4 GHz):
  PE    : 16 matmuls/group, hact (128x128) stationary, Wfc
          streams 512 -> psum (128t, 2x512v)                 (~222 us)
  GPSIMD: broadcast add tmp[h,(2u,t)] = peb[h,t] + pd[h,u]
          for h-blocks 1..3 only                             (~180 us)
  ACT   : bias-fused gelu for h-block 0 (2 instrs) + one big
          gelu over h-blocks 1..3 -> hact bf16               (~167 us)
  DVE   : psum (128,1024) fp32 -> osb bf16                   (~160 us)
  SP    : output DMAs, 256 KiB/group                         (~94 us)

Startup: the PE clock gate (HAM) keeps the array at 1.2 GHz until it has
been busy for a full ~3.4us activity window, so ~16 dummy matmuls on
zeroed scratch run during the input-DMA shadow to open the gate before
the real stream begins; group 0's gelus are split into 128-col slices so
the first output matmuls trail the ACT gelu-table load by <1us. The tail
drains the final psum tile as four quarter-chunks on three DMA queues.
"""

import sys

import numpy as np

sys.path.insert(0, "/opt/trn_rl_repo")

import ml_dtypes

import concourse.bacc as bacc
import concourse.bass as bass
import concourse.mybir as mybir
import concourse.tile as tile
from concourse.bass_utils import run_bass_kernel_spmd

B, T, U, D, H, V = 4, 512, 128, 256, 512, 512
NCORES = 8
TC = (B * T) // NCORES  # 256 t-rows per core
UB = 2  # u's per main-loop group
NG = U // UB
NWARM = 16  # PE-prewarm dummy matmuls (N=256 each, ~3.4us cold)

_PROGRAM = None
LAST_RESULT = None


def _build():
    global _PROGRAM
    if _PROGRAM is not None:
        return _PROGRAM

    f32 = mybir.dt.float32
    bf16 = mybir.dt.bfloat16
    # Bacc (not raw Bass): its compile() pipeline moves matmul waits onto
    # ldweights and splits >1-wait instructions via event semaphores —
    # walrus rejects matmuls carrying 2 sync waits otherwise.
    nc = bacc.Bacc("TRN2", target_bir_lowering=False)

    # Host-precomputed projections, pre-tiled to partition-major layouts:
    # peb[p, ht*TC+t] = (enc@We)[t, ht*128+p];  pd[p, ht*U+u] includes b1.
    peb_d = nc.declare_dram_parameter("peb", (128, 4 * TC), bf16, isOutput=False)
    pd_d = nc.declare_dram_parameter("pd", (128, 4 * U), bf16, isOutput=False)
    wfc_d = nc.declare_dram_parameter("Wfc", (128, 4 * V), bf16, isOutput=False)
    out_d = nc.declare_dram_parameter("out", (TC, U, V), bf16, isOutput=True)

    GELU = mybir.ActivationFunctionType.Gelu_apprx_tanh

    with tile.TileContext(nc) as tc:
        with (
            tc.tile_pool(name="const", bufs=1) as cpool,
            tc.tile_pool(name="tmps", bufs=3) as tpool,
            tc.tile_pool(name="hacts", bufs=3) as hpool,
            tc.tile_pool(name="outsb", bufs=6) as osb_pool,
        ):
            peb_sb = cpool.tile([128, 4 * TC], bf16)
            pd_sb = cpool.tile([128, 4 * U], bf16)
            wfc_sb = cpool.tile([128, 4 * V], bf16)  # block ht = Wfc[ht*128:...]
            warm_sb = cpool.tile([128, 256], bf16)  # PE-prewarm scratch
            tldummy_sb = cpool.tile([128, 1], f32)  # gelu-table-preload sink

            # The 16 DMA engines are shared across queues, so ARRIVAL ORDER
            # (earliest-needed-first) is what matters: peb+pd (gelu inputs)
            # first, then wfc in per-ht chunks — sub-range dep tracking lets
            # the ht-k output matmuls start as each chunk lands.
            nc.vector.memset(warm_sb, 0)
            nc.sync.dma_start(peb_sb, peb_d[:, :])
            nc.scalar.dma_start(pd_sb, pd_d[:, :])
            # Dummy gelu right after the pd DMA issue: forces the lazily
            # emitted ACT gelu-table load (~1.3us) to run during the input
            # DMA shadow. Otherwise the scheduler parks a peb-DMA semaphore
            # wait ahead of it and the table load lands on the critical
            # path of the first real gelu.
            nc.scalar.activation(tldummy_sb, warm_sb[:, 0:1], GELU)
            nc.gpsimd.dma_start(wfc_sb[:, 0:V], wfc_d[:, 0:V])
            nc.gpsimd.dma_start(wfc_sb[:, V : 2 * V], wfc_d[:, V : 2 * V])
            nc.sync.dma_start(wfc_sb[:, 2 * V : 3 * V], wfc_d[:, 2 * V : 3 * V])
            nc.sync.dma_start(wfc_sb[:, 3 * V :], wfc_d[:, 3 * V :])

            # HAM prewarm: dummy matmuls on zeroed scratch keep the PE busy
            # through its 4096-cycle activity window while the input DMAs
            # stream, so the clock gate opens to 8/8 (2.4 GHz) right as the
            # real stream begins. The N=128 tail gives finer granularity at
            # the handoff so real matmuls aren't queued behind a long dummy.
            with tc.tile_pool(name="warm_ps", bufs=1, space="PSUM") as wpool:
                warm_ps = wpool.tile([128, 256], f32)
                for _ in range(NWARM):
                    nc.tensor.matmul(
                        warm_ps, warm_sb[:, :128], warm_sb, start=True, stop=True
                    )
                for _ in range(6):
                    nc.tensor.matmul(
                        warm_ps[:, :128],
                        warm_sb[:, :128],
                        warm_sb[:, :128],
                        start=True,
                        stop=True,
                    )

            # Broadcast-add source APs for h-blocks 1..3, iteration order
            # (u, ht, t): peb u-dim stride 0; pd t-dim stride 0.
            peb_bc = (
                peb_sb[:, TC : 4 * TC]
                .rearrange("p (i t) -> p i t", i=3)
                .unsqueeze(1)
                .broadcast_to((128, UB, 3, TC))
            )
            pd_iu = pd_sb.rearrange("p (i u) -> p i u", i=4)

            # Main loop over groups of UB u's.
            out_ps_pool = tc.alloc_tile_pool(name="out_ps", bufs=8, space="PSUM")
            for g in range(NG):
                u0 = g * UB
                hact = hpool.tile([128, UB * 4 * TC], bf16, tag="hact")
                if g < 1:
                    # First group: all h-blocks via ACT bias-fused gelu, in
                    # 128-col slices ordered (ts, ui, ht) so the first
                    # output matmuls start after just 4 small gelus.
                    for ts in range(TC // 128):
                        for ui in range(UB):
                            for ht in range(4):
                                nc.scalar.activation(
                                    hact[
                                        :,
                                        ui * 4 * TC
                                        + ht * TC
                                        + ts * 128 : ui * 4 * TC
                                        + ht * TC
                                        + ts * 128
                                        + 128,
                                    ],
                                    peb_sb[:, ht * TC + ts * 128 : ht * TC + ts * 128 + 128],
                                    GELU,
                                    bias=pd_sb[:, ht * U + u0 + ui : ht * U + u0 + ui + 1],
                                )
                else:
                    # h-block 0: gelu straight from peb with pd as
                    # per-partition bias — skips the explicit add.
                    for ui in range(UB):
                        nc.scalar.activation(
                            hact[:, ui * 4 * TC : ui * 4 * TC + TC],
                            peb_sb[:, 0:TC],
                            GELU,
                            bias=pd_sb[:, u0 + ui : u0 + ui + 1],
                        )
                    # h-blocks 1..3: GPSIMD broadcast add, then one big gelu.
                    tmp = tpool.tile([128, UB * 3 * TC], f32, tag="tmp")
                    pd_bc = (
                        pd_iu[:, 1:4, u0 : u0 + UB]
                        .transpose([0, 2, 1])
                        .unsqueeze(3)
                        .broadcast_to((128, UB, 3, TC))
                    )
                    nc.gpsimd.tensor_tensor(
                        tmp.rearrange("p (u i t) -> p u i t", u=UB, i=3),
                        peb_bc,
                        pd_bc,
                        mybir.AluOpType.add,
                    )
                    nc.scalar.activation(
                        hact.rearrange("p (u x) -> p u x", u=UB)[:, :, TC : 4 * TC],
                        tmp.rearrange("p (u x) -> p u x", u=UB),
                        GELU,
                    )

                # psum tiles are one bank each (128 t, 512 v) per (ts, ui):
                # PE writes and DVE reads serialize within a bank, so
                # bank-granular tiles let chunk k+1's matmuls overlap chunk
                # k's drain — in steady state AND in the final-group tail.
                last = g == NG - 1
                qs = [nc.sync, nc.scalar, nc.gpsimd, nc.sync]
                for ts in range(TC // 128):
                    osb = None
                    if not last:
                        osb = osb_pool.tile([128, UB * V], bf16)
                    for ui in range(UB):
                        final = last and ts == TC // 128 - 1 and ui == UB - 1
                        if final:
                            # Very last chunk: two half-V pieces in separate
                            # psum tiles (separate banks — PE-write vs
                            # DVE-read serialize within a bank) so only a
                            # half-size cast + DMA chain trails the last
                            # matmul.
                            for vh in range(2):
                                ops = out_ps_pool.tile([128, V], f32, tag="ops")
                                for ht in range(4):
                                    nc.tensor.matmul(
                                        ops[:, : V // 2],
                                        hact[
                                            :,
                                            ui * 4 * TC
                                            + ht * TC
                                            + ts * 128 : ui * 4 * TC
                                            + ht * TC
                                            + ts * 128
                                            + 128,
                                        ],
                                        wfc_sb[
                                            :,
                                            ht * V
                                            + vh * (V // 2) : ht * V
                                            + (vh + 1) * (V // 2),
                                        ],
                                        start=(ht == 0),
                                        stop=(ht == 3),
                                    )
                                osbq = osb_pool.tile([128, V // 2], bf16, name=f"osbf{vh}")
                                nc.vector.tensor_copy(osbq, ops[:, : V // 2])
                                [nc.scalar, nc.sync][vh].dma_start(
                                    out_d[
                                        ts * 128 : (ts + 1) * 128,
                                        u0 + ui : u0 + ui + 1,
                                        vh * (V // 2) : (vh + 1) * (V // 2),
                                    ],
                                    osbq[:, None, :],
                                )
                            continue
                        ops = out_ps_pool.tile([128, V], f32, tag="ops")
                        for ht in range(4):
                            nc.tensor.matmul(
                                ops,
                                hact[
                                    :,
                                    ui * 4 * TC
                                    + ht * TC
                                    + ts * 128 : ui * 4 * TC
                                    + ht * TC
                                    + ts * 128
                                    + 128,
                                ],
                                wfc_sb[:, ht * V : (ht + 1) * V],
                                start=(ht == 0),
                                stop=(ht == 3),
                            )
                        if last:
                            # Final group: per-chunk osb tiles + one DMA
                            # queue per chunk so the four drains pipeline
                            # and only the last chunk's short chain
                            # (cast + DMA) sits exposed in the tail.
                            qi = ts * UB + ui
                            osbq = osb_pool.tile([128, V], bf16, name=f"osbq{qi}")
                            nc.vector.tensor_copy(osbq, ops)
                            qs[qi].dma_start(
                                out_d[
                                    ts * 128 : (ts + 1) * 128, u0 + ui : u0 + ui + 1, :
                                ],
                                osbq[:, None, :],
                            )
                        else:
                            nc.vector.tensor_copy(osb[:, ui * V : (ui + 1) * V], ops)
                    if not last:
                        nc.sync.dma_start(
                            out_d[ts * 128 : (ts + 1) * 128, u0 : u0 + UB, :],
                            osb.rearrange("p (u v) -> p u v", u=UB),
                        )
            out_ps_pool.release()

    nc.compile()
    _PROGRAM = nc
    return nc


def kernel(enc, dec, W1, b1, Wfc):
    global LAST_RESULT
    nc = _build()
    bf = ml_dtypes.bfloat16
    enc = np.asarray(enc, dtype=np.float32)
    dec = np.asarray(dec, dtype=np.float32)
    W1 = np.asarray(W1, dtype=np.float32)
    b1 = np.asarray(b1, dtype=np.float32)
    Wfc = np.asarray(Wfc, dtype=np.float32)

    # Pre-tile to partition-major (128, free) SBUF layouts.
    def pmaj(x, nblk):  # (nblk*128, F) -> (128, nblk*F)
        F = x.shape[1]
        return np.ascontiguousarray(
            x.reshape(nblk, 128, F).transpose(1, 0, 2).reshape(128, nblk * F)
        )

    wfct = pmaj(Wfc, 4).astype(bf)
    We, Wd = W1[:D], W1[D:]

    in_maps = []
    for c in range(NCORES):
        b, t0 = c // 2, (c % 2) * TC
        pe = enc[b, t0 : t0 + TC, :] @ We  # (TC, H)
        pd = dec[b] @ Wd + b1  # (U, H)
        in_maps.append(
            {
                "peb": pmaj(np.ascontiguousarray(pe.T), 4).astype(bf),
                "pd": pmaj(np.ascontiguousarray(pd.T), 4).astype(bf),
                "Wfc": wfct,
            }
        )

    LAST_RESULT = run_bass_kernel_spmd(nc, in_maps, list(range(NCORES)))

    out = np.empty((B, T, U, V), np.float32)
    for c in range(NCORES):
        b, t0 = c // 2, (c % 2) * TC
        out[b, t0 : t0 + TC] = LAST_RESULT.results[c]["out"].astype(np.float32)
    return out


# revision 16
# speedup vs baseline: 1.0031x; 1.0031x over previous
"""RNN-T JointNet kernel for 8 Trainium2 NeuronCores.

out[b,t,u,:] = gelu_tanh(enc[b,t]@We + dec[b,u]@Wd + b1) @ Wfc

Sharding: flatten (B=4, T=512) -> 2048 rows, 256 contiguous rows per core.
Core c handles batch b=c//2, time slice t0=(c%2)*256 .. +256.

The tiny projections pe=enc@We and pd=dec@Wd+b1 (<1% of FLOPs) are
precomputed on host and shipped as bf16, so the device kernel is just
gelu(pe[t]+pd[u]) @ Wfc. This halves the input bytes on the startup
critical path (the 16 DMA engines are shared across queues, so input
loading is bandwidth-bound at ~400 GB/s aggregate) and removes the
on-device prologue matmuls/evacuations entirely.

Mixed precision: the fc matmul dominates (32768x512x512 per core) and fp32
matmuls run at 1/4 PE rate, so hact and Wfc are bf16 (1 col/cycle). The
gelu input stays fp32. Output is stored bf16 (halves the 512 MiB HBM
write) and upcast on host. Norm rel err ~3.8e-3, well under the 2e-2 gate.

Per-core engine budget @64 groups of 2 u's (PE is the floor: 1024 bf16
matmuls x 512 moving cols = 218.5 us streaming at 2.4 GHz):
  PE    : 16 matmuls/group, hact (128x128) stationary, Wfc
          streams 512 -> psum (128t, 2x512v)                 (~222 us)
  GPSIMD: broadcast add tmp[h,(2u,t)] = peb[h,t] + pd[h,u]
          for h-blocks 1..3 only                             (~180 us)
  ACT   : bias-fused gelu for h-block 0 (2 instrs) + one big
          gelu over h-blocks 1..3 -> hact bf16               (~167 us)
  DVE   : psum (128,1024) fp32 -> osb bf16                   (~160 us)
  SP    : output DMAs, 256 KiB/group                         (~94 us)

Startup: the PE clock gate (HAM) keeps the array at 1.2 GHz until it has
been busy for a full ~3.4us activity window, so ~16 dummy matmuls on
zeroed scratch run during the input-DMA shadow to open the gate before
the real stream begins; group 0's gelus are split into 128-col slices so
the first output matmuls trail the ACT gelu-table load by <1us. The tail
drains the final psum tile as four quarter-chunks on three DMA queues.
"""

import sys

import numpy as np

sys.path.insert(0, "/opt/trn_rl_repo")

import ml_dtypes

import concourse.bacc as bacc
import concourse.bass as bass
import concourse.mybir as mybir
import concourse.tile as tile
from concourse.bass_utils import run_bass_kernel_spmd

B, T, U, D, H, V = 4, 512, 128, 256, 512, 512
NCORES = 8
TC = (B * T) // NCORES  # 256 t-rows per core
UB = 2  # u's per main-loop group
NG = U // UB
NWARM = 16  # PE-prewarm dummy matmuls (N=256 each, ~3.4us cold)

_PROGRAM = None
LAST_RESULT = None


def _build():
    global _PROGRAM
    if _PROGRAM is not None:
        return _PROGRAM

    f32 = mybir.dt.float32
    bf16 = mybir.dt.bfloat16
    # Bacc (not raw Bass): its compile() pipeline moves matmul waits onto
    # ldweights and splits >1-wait instructions via event semaphores —
    # walrus rejects matmuls carrying 2 sync waits otherwise.
    nc = bacc.Bacc("TRN2", target_bir_lowering=False)

    # Host-precomputed projections, pre-tiled to partition-major layouts:
    # peb[p, ht*TC+t] = (enc@We)[t, ht*128+p];  pd[p, ht*U+u] includes b1.
    peb_d = nc.declare_dram_parameter("peb", (128, 4 * TC), bf16, isOutput=False)
    pd_d = nc.declare_dram_parameter("pd", (128, 4 * U), bf16, isOutput=False)
    wfc_d = nc.declare_dram_parameter("Wfc", (128, 4 * V), bf16, isOutput=False)
    out_d = nc.declare_dram_parameter("out", (TC, U, V), bf16, isOutput=True)

    GELU = mybir.ActivationFunctionType.Gelu_apprx_tanh

    with tile.TileContext(nc) as tc:
        with (
            tc.tile_pool(name="const", bufs=1) as cpool,
            tc.tile_pool(name="tmps", bufs=3) as tpool,
            tc.tile_pool(name="hacts", bufs=3) as hpool,
            tc.tile_pool(name="outsb", bufs=6) as osb_pool,
        ):
            peb_sb = cpool.tile([128, 4 * TC], bf16)
            pd_sb = cpool.tile([128, 4 * U], bf16)
            wfc_sb = cpool.tile([128, 4 * V], bf16)  # block ht = Wfc[ht*128:...]
            warm_sb = cpool.tile([128, 256], bf16)  # PE-prewarm scratch
            tldummy_sb = cpool.tile([128, 1], f32)  # gelu-table-preload sink

            # The 16 DMA engines are shared across queues, so ARRIVAL ORDER
            # (earliest-needed-first) is what matters: peb+pd (gelu inputs)
            # first, then wfc in per-ht chunks — sub-range dep tracking lets
            # the ht-k output matmuls start as each chunk lands.
            nc.vector.memset(warm_sb, 0)
            nc.sync.dma_start(peb_sb, peb_d[:, :])
            nc.scalar.dma_start(pd_sb, pd_d[:, :])
            # Dummy gelu right after the pd DMA issue: forces the lazily
            # emitted ACT gelu-table load (~1.3us) to run during the input
            # DMA shadow. Otherwise the scheduler parks a peb-DMA semaphore
            # wait ahead of it and the table load lands on the critical
            # path of the first real gelu.
            nc.scalar.activation(tldummy_sb, warm_sb[:, 0:1], GELU)
            nc.gpsimd.dma_start(wfc_sb[:, 0:V], wfc_d[:, 0:V])
            nc.gpsimd.dma_start(wfc_sb[:, V : 2 * V], wfc_d[:, V : 2 * V])
            nc.sync.dma_start(wfc_sb[:, 2 * V : 3 * V], wfc_d[:, 2 * V : 3 * V])
            nc.sync.dma_start(wfc_sb[:, 3 * V :], wfc_d[:, 3 * V :])

            # HAM prewarm: dummy matmuls on zeroed scratch keep the PE busy
            # through its 4096-cycle activity window while the input DMAs
            # stream, so the clock gate opens to 8/8 (2.4 GHz) right as the
            # real stream begins. The N=128 tail gives finer granularity at
            # the handoff so real matmuls aren't queued behind a long dummy.
            with tc.tile_pool(name="warm_ps", bufs=1, space="PSUM") as wpool:
                warm_ps = wpool.tile([128, 256], f32)
                for _ in range(NWARM):
                    nc.tensor.matmul(
                        warm_ps, warm_sb[:, :128], warm_sb, start=True, stop=True
                    )
                for _ in range(6):
                    nc.tensor.matmul(
                        warm_ps[:, :128],
                        warm_sb[:, :128],
                        warm_sb[:, :128],
                        start=True,
                        stop=True,
                    )

            # Broadcast-add source APs for h-blocks 1..3, iteration order
            # (u, ht, t): peb u-dim stride 0; pd t-dim stride 0.
            peb_bc = (
                peb_sb[:, TC : 4 * TC]
                .rearrange("p (i t) -> p i t", i=3)
                .unsqueeze(1)
                .broadcast_to((128, UB, 3, TC))
            )
            pd_iu = pd_sb.rearrange("p (i u) -> p i u", i=4)

            # Main loop over groups of UB u's.
            out_ps_pool = tc.alloc_tile_pool(name="out_ps", bufs=8, space="PSUM")
            for g in range(NG):
                u0 = g * UB
                hact = hpool.tile([128, UB * 4 * TC], bf16, tag="hact")
                if g < 1:
                    # First group: all h-blocks via ACT bias-fused gelu, in
                    # 128-col slices ordered (ts, ui, ht) so the first
                    # output matmuls start after just 4 small gelus.
                    for ts in range(TC // 128):
                        for ui in range(UB):
                            for ht in range(4):
                                nc.scalar.activation(
                                    hact[
                                        :,
                                        ui * 4 * TC
                                        + ht * TC
                                        + ts * 128 : ui * 4 * TC
                                        + ht * TC
                                        + ts * 128
                                        + 128,
                                    ],
                                    peb_sb[:, ht * TC + ts * 128 : ht * TC + ts * 128 + 128],
                                    GELU,
                                    bias=pd_sb[:, ht * U + u0 + ui : ht * U + u0 + ui + 1],
                                )
                else:
                    # h-block 0: gelu straight from peb with pd as
                    # per-partition bias — skips the explicit add.
                    for ui in range(UB):
                        nc.scalar.activation(
                            hact[:, ui * 4 * TC : ui * 4 * TC + TC],
                            peb_sb[:, 0:TC],
                            GELU,
                            bias=pd_sb[:, u0 + ui : u0 + ui + 1],
                        )
                    # h-blocks 1..3: GPSIMD broadcast add, then one big gelu.
                    tmp = tpool.tile([128, UB * 3 * TC], f32, tag="tmp")
                    pd_bc = (
                        pd_iu[:, 1:4, u0 : u0 + UB]
                        .transpose([0, 2, 1])
                        .unsqueeze(3)
                        .broadcast_to((128, UB, 3, TC))
                    )
                    nc.gpsimd.tensor_tensor(
                        tmp.rearrange("p (u i t) -> p u i t", u=UB, i=3),
                        peb_bc,
                        pd_bc,
                        mybir.AluOpType.add,
                    )
                    nc.scalar.activation(
                        hact.rearrange("p (u x) -> p u x", u=UB)[:, :, TC : 4 * TC],
                        tmp.rearrange("p (u x) -> p u x", u=UB),
                        GELU,
                    )

                # psum tiles are one bank each (128 t, 512 v) per (ts, ui):
                # PE writes and DVE reads serialize within a bank, so
                # bank-granular tiles let chunk k+1's matmuls overlap chunk
                # k's drain — in steady state AND in the final-group tail.
                last = g == NG - 1
                qs = [nc.sync, nc.scalar, nc.gpsimd, nc.sync]
                for ts in range(TC // 128):
                    osb = None
                    if not last:
                        osb = osb_pool.tile([128, UB * V], bf16)
                    for ui in range(UB):
                        ops = out_ps_pool.tile([128, V], f32, tag="ops")
                        for ht in range(4):
                            nc.tensor.matmul(
                                ops,
                                hact[
                                    :,
                                    ui * 4 * TC
                                    + ht * TC
                                    + ts * 128 : ui * 4 * TC
                                    + ht * TC
                                    + ts * 128
                                    + 128,
                                ],
                                wfc_sb[:, ht * V : (ht + 1) * V],
                                start=(ht == 0),
                                stop=(ht == 3),
                            )
                        if last:
                            # Final group: per-chunk osb tiles + one DMA
                            # queue per chunk so the four drains pipeline
                            # and only the last chunk's short chain
                            # (cast + DMA) sits exposed in the tail.
                            qi = ts * UB + ui
                            osbq = osb_pool.tile([128, V], bf16, name=f"osbq{qi}")
                            nc.vector.tensor_copy(osbq, ops)
                            qs[qi].dma_start(
                                out_d[
                                    ts * 128 : (ts + 1) * 128, u0 + ui : u0 + ui + 1, :
                                ],
                                osbq[:, None, :],
                            )
                        else:
                            nc.vector.tensor_copy(osb[:, ui * V : (ui + 1) * V], ops)
                    if not last:
                        nc.sync.dma_start(
                            out_d[ts * 128 : (ts + 1) * 128, u0 : u0 + UB, :],
                            osb.rearrange("p (u v) -> p u v", u=UB),
                        )
            out_ps_pool.release()

    nc.compile()
    _PROGRAM = nc
    return nc


def kernel(enc, dec, W1, b1, Wfc):
    global LAST_RESULT
    nc = _build()
    bf = ml_dtypes.bfloat16
    enc = np.asarray(enc, dtype=np.float32)
    dec = np.asarray(dec, dtype=np.float32)
    W1 = np.asarray(W1, dtype=np.float32)
    b1 = np.asarray(b1, dtype=np.float32)
    Wfc = np.asarray(Wfc, dtype=np.float32)

    # Pre-tile to partition-major (128, free) SBUF layouts.
    def pmaj(x, nblk):  # (nblk*128, F) -> (128, nblk*F)
        F = x.shape[1]
        return np.ascontiguousarray(
            x.reshape(nblk, 128, F).transpose(1, 0, 2).reshape(128, nblk * F)
        )

    wfct = pmaj(Wfc, 4).astype(bf)
    We, Wd = W1[:D], W1[D:]

    in_maps = []
    for c in range(NCORES):
        b, t0 = c // 2, (c % 2) * TC
        pe = enc[b, t0 : t0 + TC, :] @ We  # (TC, H)
        pd = dec[b] @ Wd + b1  # (U, H)
        in_maps.append(
            {
                "peb": pmaj(np.ascontiguousarray(pe.T), 4).astype(bf),
                "pd": pmaj(np.ascontiguousarray(pd.T), 4).astype(bf),
                "Wfc": wfct,
            }
        )

    LAST_RESULT = run_bass_kernel_spmd(nc, in_maps, list(range(NCORES)))

    out = np.empty((B, T, U, V), np.float32)
    for c in range(NCORES):
        b, t0 = c // 2, (c % 2) * TC
        out[b, t0 : t0 + TC] = LAST_RESULT.results[c]["out"].astype(np.float32)
    return out
